# revision 20
# baseline (speedup 1.0000x reference)
"""ASR decoder (2-layer LSTM, H=1024, B=64, 127 steps) on 8 Trainium2 cores.

Strategy: gate-sharding. Each core owns 128 of the 1024 hidden units of each
LSTM layer (i.e. 512 of the 4096 gate rows), with the full batch of 64.
Per "superstep" s the wavefront computes, fully in parallel per core:
  - L0: h0[t=s]    = LSTM0(x_s, h0[s-1])        (8 fp16 matmuls + emb gather)
  - L1: h1[t=s-1]  = LSTM1(h0[s-1], h1[s-2])    (16 fp16 matmuls)
  - logits[t=s-2]  = W_out_shard @ h1[s-2]      (8 fp16 matmuls)
then one 8-core AllGather exchanges the two fresh 128-unit h-chunks
(fp16, [256,64] per rank) so every core has the full h vectors next step.

Algebraic simplifications vs the reference:
  - mean-pool commutes with the linear projection: project mean(audio) only.
  - the one-hot @ W_ih0 matmul is an embedding row-gather (indirect DMA).
dtypes: fp16 weights/activations on the PE (1 cycle/row vs 4 for fp32),
fp32 PSUM accumulation, fp32 cell state and gate activations.
"""
import numpy as np

import concourse.bacc as bacc
import concourse.bass as bass
import concourse.mybir as mybir
import concourse.tile as tile
from concourse.masks import make_identity

F32 = mybir.dt.float32
F16 = mybir.dt.float16
I32 = mybir.dt.int32
AF = mybir.ActivationFunctionType

N_CORES = 8
B = 64            # batch
T_A = 500         # audio time
D_IN = 768        # audio dim
H = 1024          # hidden
G = 512           # gate rows per core (4 gates x 128 units)
U = 128           # units per core
SEQ = 128
STEPS = SEQ - 1   # 127 LSTM steps / output positions
N_SUPER = STEPS + 2  # wavefront supersteps


def _emit(nc, n_super, dbg=False, no_cc=False, no_emb=False, no_logits=False, repeat=1, split_ag=False):
    """Emit the whole kernel body under a TileContext."""
    steps = n_super - 2  # number of time steps actually computed

    # ---------------- DRAM I/O (per core) ----------------
    KC = H // 128  # 8 contraction chunks
    audio = nc.dram_tensor("audio", [B // N_CORES, T_A, D_IN], F32, kind="ExternalInput")
    tidx = nc.dram_tensor("tidx", [B, SEQ], I32, kind="ExternalInput")
    wproj = nc.dram_tensor("wproj", [D_IN, H], F16, kind="ExternalInput")     # (W_proj/500).T
    bproj = nc.dram_tensor("bproj", [H], F32, kind="ExternalInput")
    wh0t = nc.dram_tensor("wh0t", [H, G], F16, kind="ExternalInput")          # W_hh0[rows_k].T
    wi0t = nc.dram_tensor("wi0t", [H, G], F16, kind="ExternalInput")          # W_ih0[rows_k].T
    wi1t = nc.dram_tensor("wi1t", [H, G], F16, kind="ExternalInput")          # W_ih1[rows_k].T
    wh1t = nc.dram_tensor("wh1t", [H, G], F16, kind="ExternalInput")          # W_hh1[rows_k].T
    emb0 = nc.dram_tensor("emb0", [H, G], F16, kind="ExternalInput")          # W_ih0[rows_k].T + b0
    bias0 = nc.dram_tensor("bias0", [B, G], F32, kind="ExternalInput")        # b0 broadcast
    bias1 = nc.dram_tensor("bias1", [B, G], F32, kind="ExternalInput")        # b1 broadcast
    woutt = nc.dram_tensor("woutt", [H, U], F16, kind="ExternalInput")        # W_out[rows char].T
    bout = nc.dram_tensor("bout", [U], F32, kind="ExternalInput")
    logits = nc.dram_tensor("logits", [steps, U, B], F32, kind="ExternalOutput")
    if dbg:
        d_mean = nc.dram_tensor("d_mean", [1, B // N_CORES * D_IN], F16, kind="ExternalOutput")
        d_enc = nc.dram_tensor("d_enc", [128, KC, B // N_CORES], F16, kind="ExternalOutput")
        d_x0 = nc.dram_tensor("d_x0", [128, KC, B], F16, kind="ExternalOutput")
        d_h0c0 = nc.dram_tensor("d_h0c0", [128, B], F16, kind="ExternalOutput")
        d_h0s0 = nc.dram_tensor("d_h0s0", [128, KC, B], F16, kind="ExternalOutput")
        d_h1s1 = nc.dram_tensor("d_h1s1", [128, KC, B], F16, kind="ExternalOutput")
        d_emb1 = nc.dram_tensor("d_emb1", [B, G], F16, kind="ExternalOutput")
        d_g0s1 = nc.dram_tensor("d_g0s1", [B, G], F32, kind="ExternalOutput")

    with tile.TileContext(nc) as tc:
        with (
            tc.tile_pool(name="wpool", bufs=1) as wpool,
            tc.tile_pool(name="state", bufs=1) as state,
            tc.tile_pool(name="dram", bufs=1, space="DRAM") as dpool,
            tc.tile_pool(name="hpool", bufs=2) as hpool,
            tc.tile_pool(name="gpool", bufs=3) as gpool,
            tc.tile_pool(name="psg", bufs=2, space="PSUM") as psg,
            tc.tile_pool(name="pst", bufs=2, space="PSUM") as pst,
            tc.tile_pool(name="psl", bufs=2, space="PSUM") as psl,
        ):
            # ---------------- persistent SBUF ----------------
            wh0_sb = wpool.tile([128, KC, G], F16, name="wh0_sb")
            wi1_sb = wpool.tile([128, KC, G], F16, name="wi1_sb")
            wh1_sb = wpool.tile([128, KC, G], F16, name="wh1_sb")
            wout_sb = wpool.tile([128, KC, U], F16, name="wout_sb")
            bias0_sb = wpool.tile([B, G], F32, name="bias0_sb")
            bias1_sb = wpool.tile([B, G], F32, name="bias1_sb")
            bout_sb = wpool.tile([U, 1], F32, name="bout_sb")
            idx_sb = wpool.tile([B, SEQ], I32, name="idx_sb")
            ident = wpool.tile([128, 128], F16, name="ident")
            c0_sb = state.tile([B, U], F32, name="c0_sb")
            c1_sb = state.tile([B, U], F32, name="c1_sb")

            nc.sync.dma_start(wh0_sb[:], wh0t.ap().rearrange("(c p) g -> p c g", p=128))
            nc.sync.dma_start(wi1_sb[:], wi1t.ap().rearrange("(c p) g -> p c g", p=128))
            nc.sync.dma_start(wh1_sb[:], wh1t.ap().rearrange("(c p) g -> p c g", p=128))
            nc.sync.dma_start(wout_sb[:], woutt.ap().rearrange("(c p) u -> p c u", p=128))
            nc.sync.dma_start(bias0_sb[:], bias0.ap())
            nc.sync.dma_start(bias1_sb[:], bias1.ap())
            nc.sync.dma_start(bout_sb[:], bout.ap().rearrange("(u one) -> u one", one=1))
            nc.sync.dma_start(idx_sb[:], tidx.ap())
            make_identity(nc, ident[:])
            nc.gpsimd.memset(c0_sb[:], 0.0)
            nc.gpsimd.memset(c1_sb[:], 0.0)

            # collective buffers
            cc_in = dpool.tile([2 * 128, B], F16, name="cc_in", bufs=2)
            cc_out = dpool.tile([N_CORES * 2 * 128, B], F16, name="cc_out",
                                addr_space="Shared", bufs=2)

            # =============== prologue: audio mean + projection ===============
            with (
                tc.tile_pool(name="apool", bufs=2) as apool,
                tc.tile_pool(name="appsum", bufs=1, space="PSUM") as appsum,
                tc.tile_pool(name="prpool", bufs=1) as prpool,
            ):
                ones_sb = prpool.tile([128, 1], F16, name="ones_sb")
                nc.gpsimd.memset(ones_sb[:], 1.0)
                wproj_sb = prpool.tile([128, 6, H], F16, name="wproj_sb")
                nc.sync.dma_start(wproj_sb[:], wproj.ap().rearrange("(c p) h -> p c h", p=128))
                bproj_sb = prpool.tile([128, KC], F32, name="bproj_sb")
                nc.sync.dma_start(bproj_sb[:], bproj.ap().rearrange("(c p) -> p c", p=128))

                # audio sum over time: 4 chunks of <=128 time rows
                a_t = audio.ap().rearrange("b t d -> t b d")
                tchunks = [(0, 128), (128, 128), (256, 128), (384, 116)]
                a16s = []
                for (t0, tcnt) in tchunks:
                    a32 = apool.tile([128, B // N_CORES, D_IN], F32, name="a32")
                    nc.sync.dma_start(a32[:tcnt], a_t[t0:t0 + tcnt])
                    a16 = gpool.tile([128, B // N_CORES * D_IN], F16, name="a16",
                                     tag="a16", bufs=4)
                    nc.scalar.activation(a16[:tcnt], a32[:tcnt].rearrange("p b d -> p (b d)"),
                                         AF.Copy)
                    a16s.append((a16, tcnt))
                # ones-matmul reduce: psum [1, 1024] per group (2 banks)
                mean16 = prpool.tile([1, B // N_CORES * D_IN], F16, name="mean16")
                for grp in range(6):
                    ps_m = appsum.tile([1, 1024], F32, name="ps_m", tag="ps_m")
                    for nn in range(2):
                        o = grp * 1024 + nn * 512
                        for ti, (a16, tcnt) in enumerate(a16s):
                            nc.tensor.matmul(
                                ps_m[:, nn * 512:(nn + 1) * 512],
                                ones_sb[:tcnt, :],
                                a16[:tcnt, o: o + 512],
                                start=(ti == 0), stop=(ti == 3),
                            )
                    nc.scalar.activation(mean16[:, grp * 1024:(grp + 1) * 1024],
                                         ps_m[:], AF.Copy)
                # redistribute [1, 8*768] -> [8, 768] via DRAM scratch
                mean_dr = dpool.tile([B // N_CORES, D_IN], F16, name="mean_dr")
                nc.sync.dma_start(mean_dr[:].rearrange("b d -> (b d)")[None, :], mean16[:])
                mean8 = prpool.tile([B // N_CORES, D_IN], F16, name="mean8")
                nc.sync.dma_start(mean8[:], mean_dr[:])
                # transpose to [d, b]: 6 PE transposes of [8, 128]
                meanT = prpool.tile([128, 6, B // N_CORES], F16, name="meanT")
                for dc in range(6):
                    ps_t = pst.tile([128, B], F16, name="ps_t", tag="t")
                    nc.tensor.transpose(ps_t[:, :8], mean8[:, dc * 128:(dc + 1) * 128],
                                        ident[:8, :8])
                    nc.vector.tensor_copy(meanT[:, dc, :], ps_t[:, :8])
                # projection: enc_k [h, 8] = W_proj' @ mean_k + b_proj
                enc_sb = prpool.tile([128, KC, B // N_CORES], F16, name="enc_sb")
                for hc in range(KC):
                    ps_p = psl.tile([128, B], F32, name="ps_p", tag="l")
                    for dc in range(6):
                        nc.tensor.matmul(ps_p[:, :8],
                                         wproj_sb[:, dc, hc * 128:(hc + 1) * 128],
                                         meanT[:, dc, :],
                                         start=(dc == 0), stop=(dc == 5))
                    nc.scalar.activation(enc_sb[:, hc, :], ps_p[:, :8], AF.Identity,
                                         bias=bproj_sb[:, hc:hc + 1])
                enc_dram = dpool.tile([H, B // N_CORES], F16, name="enc_dram")
                nc.sync.dma_start(enc_dram[:].rearrange("(c p) b -> p c b", p=128), enc_sb[:])
                x0_sb = prpool.tile([128, KC, B], F16, name="x0_sb")
                if no_cc:
                    nc.gpsimd.memset(x0_sb[:], 0.01)
                else:
                    enc_all = dpool.tile([N_CORES * H, B // N_CORES], F16, name="enc_all",
                                         addr_space="Shared")
                    nc.gpsimd.collective_compute(
                        "AllGather", mybir.AluOpType.bypass,
                        replica_groups=[list(range(N_CORES))],
                        ins=[enc_dram.opt()], outs=[enc_all.opt()],
                    )
                    # readback x0 stationary chunks [128, hc, B]
                    ea = enc_all[:].rearrange("(r c p) b -> c p r b", r=N_CORES, p=128)
                    for hc in range(KC):
                        nc.sync.dma_start(
                            x0_sb[:, hc, :].rearrange("p (r b) -> p r b", r=N_CORES),
                            ea[hc])
                # t=0 input weights
                wi0_sb = prpool.tile([128, KC, G], F16, name="wi0_sb")
                nc.sync.dma_start(wi0_sb[:], wi0t.ap().rearrange("(c p) g -> p c g", p=128))

                if dbg:
                    nc.sync.dma_start(d_mean.ap(), mean16[:])
                    nc.sync.dma_start(d_enc.ap(), enc_sb[:])
                    nc.sync.dma_start(d_x0.ap(), x0_sb[:])
                # zero tile for the h1 slot of the first AllGather
                zero16 = prpool.tile([128, B], F16, name="zero16")
                nc.gpsimd.memset(zero16[:], 0.0)

                # =============== main wavefront loop ===============
                h0_stat = None
                h1_stat = None
                for rep, s in [(rp, sp) for rp in range(repeat)
                               for sp in range(n_super)]:
                    # ---- A) layer 0, t = s ----
                    if s <= steps - 1:
                        ps_g0 = psg.tile([B, G], F32, name="ps_g0", tag="g")
                        if s == 0:
                            for c in range(KC):
                                nc.tensor.matmul(ps_g0[:], x0_sb[:, c, :],
                                                 wi0_sb[:, c, :],
                                                 start=(c == 0), stop=(c == KC - 1))
                        else:
                            for c in range(KC):
                                nc.tensor.matmul(ps_g0[:], h0_stat[:, c, :],
                                                 wh0_sb[:, c, :],
                                                 start=(c == 0), stop=(c == KC - 1))
                        g0_sb = gpool.tile([B, G], F32, name="g0_sb", tag="g0")
                        if s == 0 or no_emb:
                            nc.vector.tensor_add(g0_sb[:], ps_g0[:], bias0_sb[:])
                        else:
                            embt = gpool.tile([B, G], F16, name="embt", tag="emb")
                            nc.gpsimd.indirect_dma_start(
                                out=embt[:], out_offset=None, in_=emb0.ap(),
                                in_offset=bass.IndirectOffsetOnAxis(
                                    ap=idx_sb[:, s:s + 1], axis=0),
                            )
                            nc.vector.tensor_add(g0_sb[:], ps_g0[:], embt[:])
                            if dbg and s == 1:
                                nc.sync.dma_start(d_emb1.ap(), embt[:])
                                nc.sync.dma_start(d_g0s1.ap(), g0_sb[:])
                        # activations: cols [i(128) f(128) o(128) g(128)]
                        a0 = gpool.tile([B, G], F32, name="a0", tag="a0")
                        nc.scalar.activation(a0[:, 0:384], g0_sb[:, 0:384], AF.Sigmoid)
                        nc.scalar.activation(a0[:, 384:512], g0_sb[:, 384:512], AF.Tanh)
                        t1 = gpool.tile([B, U], F32, name="t1", tag="t1")
                        t2 = gpool.tile([B, U], F32, name="t2", tag="t2")
                        nc.vector.tensor_mul(t1[:], a0[:, 0:128], a0[:, 384:512])
                        nc.vector.tensor_mul(t2[:], a0[:, 128:256], c0_sb[:])
                        nc.vector.tensor_add(c0_sb[:], t1[:], t2[:])
                        tc0 = gpool.tile([B, U], F32, name="tc0", tag="tc0")
                        nc.scalar.activation(tc0[:], c0_sb[:], AF.Tanh)
                        h0T = gpool.tile([B, U], F16, name="h0T", tag="h0T")
                        nc.vector.tensor_mul(h0T[:], a0[:, 256:384], tc0[:])
                        ps_t0 = pst.tile([128, B], F16, name="ps_t0", tag="t")
                        nc.tensor.transpose(ps_t0[:, :B], h0T[:], ident[:B, :B])
                        h0_chunk = gpool.tile([128, B], F16, name="h0_chunk", tag="h0c")
                        nc.vector.tensor_copy(h0_chunk[:], ps_t0[:, :B])
                        if dbg and s == 0:
                            nc.sync.dma_start(d_h0c0.ap(), h0_chunk[:])

                    # ---- B) layer 1, t = s-1 ----
                    if 1 <= s <= steps:
                        ps_g1 = psg.tile([B, G], F32, name="ps_g1", tag="g")
                        for c in range(KC):
                            nc.tensor.matmul(ps_g1[:], h0_stat[:, c, :],
                                             wi1_sb[:, c, :],
                                             start=(c == 0),
                                             stop=(c == KC - 1 and s == 1))
                        if s >= 2:
                            for c in range(KC):
                                nc.tensor.matmul(ps_g1[:], h1_stat[:, c, :],
                                                 wh1_sb[:, c, :],
                                                 start=False, stop=(c == KC - 1))
                        g1_sb = gpool.tile([B, G], F32, name="g1_sb", tag="g1")
                        nc.vector.tensor_add(g1_sb[:], ps_g1[:], bias1_sb[:])
                        a1 = gpool.tile([B, G], F32, name="a1", tag="a1")
                        nc.scalar.activation(a1[:, 0:384], g1_sb[:, 0:384], AF.Sigmoid)
                        nc.scalar.activation(a1[:, 384:512], g1_sb[:, 384:512], AF.Tanh)
                        t3 = gpool.tile([B, U], F32, name="t3", tag="t3")
                        t4 = gpool.tile([B, U], F32, name="t4", tag="t4")
                        nc.vector.tensor_mul(t3[:], a1[:, 0:128], a1[:, 384:512])
                        nc.vector.tensor_mul(t4[:], a1[:, 128:256], c1_sb[:])
                        nc.vector.tensor_add(c1_sb[:], t3[:], t4[:])
                        tc1 = gpool.tile([B, U], F32, name="tc1", tag="tc1")
                        nc.scalar.activation(tc1[:], c1_sb[:], AF.Tanh)
                        h1T = gpool.tile([B, U], F16, name="h1T", tag="h1T")
                        nc.vector.tensor_mul(h1T[:], a1[:, 256:384], tc1[:])
                        ps_t1 = pst.tile([128, B], F16, name="ps_t1", tag="t")
                        nc.tensor.transpose(ps_t1[:, :B], h1T[:], ident[:B, :B])
                        h1_chunk = gpool.tile([128, B], F16, name="h1_chunk", tag="h1c")
                        nc.vector.tensor_copy(h1_chunk[:], ps_t1[:, :B])

                    # ---- C) logits, t = s-2 ----
                    if 2 <= s <= steps + 1 and not no_logits:
                        ps_lg = psl.tile([128, B], F32, name="ps_lg", tag="l")
                        for c in range(KC):
                            nc.tensor.matmul(ps_lg[:, :B], wout_sb[:, c, :],
                                             h1_stat[:, c, :],
                                             start=(c == 0), stop=(c == KC - 1))
                        lg_sb = gpool.tile([U, B], F32, name="lg_sb", tag="lg")
                        nc.scalar.activation(lg_sb[:], ps_lg[:, :B], AF.Identity,
                                             bias=bout_sb[:, 0:1])
                        nc.sync.dma_start(logits[s - 2], lg_sb[:])

                    # ---- E/F split: per-layer AllGather + readback ----
                    if split_ag:
                        if s <= steps - 1:
                            cc0_in = dpool.tile([128, B], F16, name="cc0_in",
                                                tag="cc0_in", bufs=2)
                            nc.sync.dma_start(cc0_in[:], h0_chunk[:])
                            cc0_out = dpool.tile([N_CORES * 128, B], F16,
                                                 name="cc0_out", tag="cc0_out",
                                                 addr_space="Shared", bufs=2)
                            if not no_cc:
                                nc.gpsimd.collective_compute(
                                    "AllGather", mybir.AluOpType.bypass,
                                    replica_groups=[list(range(N_CORES))],
                                    ins=[cc0_in.opt()], outs=[cc0_out.opt()],
                                )
                            co0 = cc0_out[:].rearrange("(r p) b -> p r b", r=N_CORES)
                            h0_new = hpool.tile([128, KC, B], F16, name="h0_new",
                                                tag="h0s")
                            for q in range(4):
                                nc.sync.dma_start(h0_new[:, 2 * q:2 * q + 2, :],
                                                  co0[:, 2 * q:2 * q + 2, :])
                            h0_stat = h0_new
                        if 1 <= s <= steps:
                            cc1_in = dpool.tile([128, B], F16, name="cc1_in",
                                                tag="cc1_in", bufs=2)
                            nc.sync.dma_start(cc1_in[:], h1_chunk[:])
                            cc1_out = dpool.tile([N_CORES * 128, B], F16,
                                                 name="cc1_out", tag="cc1_out",
                                                 addr_space="Shared", bufs=2)
                            if not no_cc:
                                nc.gpsimd.collective_compute(
                                    "AllGather", mybir.AluOpType.bypass,
                                    replica_groups=[list(range(N_CORES))],
                                    ins=[cc1_in.opt()], outs=[cc1_out.opt()],
                                )
                            co1 = cc1_out[:].rearrange("(r p) b -> p r b", r=N_CORES)
                            h1_new = hpool.tile([128, KC, B], F16, name="h1_new",
                                                tag="h1s")
                            for q in range(4):
                                nc.sync.dma_start(h1_new[:, 2 * q:2 * q + 2, :],
                                                  co1[:, 2 * q:2 * q + 2, :])
                            h1_stat = h1_new
                        continue
                    if s <= steps:
                        cc_in_s = dpool.tile([2 * 128, B], F16, name="cc_in",
                                             tag="cc_in", bufs=2)
                        if s <= steps - 1:
                            nc.sync.dma_start(cc_in_s[0:128], h0_chunk[:])
                        else:
                            nc.sync.dma_start(cc_in_s[0:128], zero16[:])
                        if s == 0:
                            nc.sync.dma_start(cc_in_s[128:256], zero16[:])
                        elif s >= 1:
                            nc.sync.dma_start(cc_in_s[128:256], h1_chunk[:])
                        cc_out_s = dpool.tile([N_CORES * 2 * 128, B], F16,
                                              name="cc_out", tag="cc_out",
                                              addr_space="Shared", bufs=2)
                        if not no_cc:
                            nc.gpsimd.collective_compute(
                                "AllGather", mybir.AluOpType.bypass,
                                replica_groups=[list(range(N_CORES))],
                                ins=[cc_in_s.opt()], outs=[cc_out_s.opt()],
                            )
                        # ---- F) readback ----
                        co = cc_out_s[:].rearrange("(r l p) b -> l p r b",
                                                     r=N_CORES, l=2, p=128)
                        h0_new = hpool.tile([128, KC, B], F16, name="h0_new", tag="h0s")
                        h1_new = hpool.tile([128, KC, B], F16, name="h1_new", tag="h1s")
                        if s <= steps - 1:
                            for q in range(4):
                                nc.sync.dma_start(h0_new[:, 2 * q:2 * q + 2, :],
                                                  co[0][:, 2 * q:2 * q + 2, :])
                        if s >= 1:
                            for q in range(4):
                                nc.sync.dma_start(h1_new[:, 2 * q:2 * q + 2, :],
                                                  co[1][:, 2 * q:2 * q + 2, :])
                        if dbg and s == 0:
                            nc.sync.dma_start(d_h0s0.ap(), h0_new[:])
                        if dbg and s == 1:
                            nc.sync.dma_start(d_h1s1.ap(), h1_new[:])
                        h0_stat = h0_new
                        h1_stat = h1_new

    nc.compile()
    return nc


# ----------------------------------------------------------------------------
# host side
# ----------------------------------------------------------------------------
def _prep_in_maps(audio_features, text_indices, W_proj, b_proj,
                  W_ih0, W_hh0, b_ih0, b_hh0,
                  W_ih1, W_hh1, b_ih1, b_hh1,
                  W_out, b_out):
    audio = np.ascontiguousarray(np.asarray(audio_features, np.float32))
    tidx = np.ascontiguousarray(np.asarray(text_indices).astype(np.int32))
    wproj_t = np.ascontiguousarray((np.asarray(W_proj, np.float32) / T_A).T
                                   .astype(np.float16))
    bproj = np.asarray(b_proj, np.float32)
    b0 = np.asarray(b_ih0, np.float32) + np.asarray(b_hh0, np.float32)
    b1 = np.asarray(b_ih1, np.float32) + np.asarray(b_hh1, np.float32)
    bout_f = np.asarray(b_out, np.float32)

    in_maps = []
    for k in range(N_CORES):
        u = slice(128 * k, 128 * (k + 1))
        # gate column order [i | f | o | g]  (pytorch rows: i, f, g, o)
        rows = np.concatenate([
            np.arange(128 * k, 128 * (k + 1)),            # i
            H + np.arange(128 * k, 128 * (k + 1)),        # f
            3 * H + np.arange(128 * k, 128 * (k + 1)),    # o
            2 * H + np.arange(128 * k, 128 * (k + 1)),    # g
        ])
        wh0t = np.ascontiguousarray(np.asarray(W_hh0, np.float32)[rows].T.astype(np.float16))
        wi0t = np.ascontiguousarray(np.asarray(W_ih0, np.float32)[rows].T.astype(np.float16))
        wi1t = np.ascontiguousarray(np.asarray(W_ih1, np.float32)[rows].T.astype(np.float16))
        wh1t = np.ascontiguousarray(np.asarray(W_hh1, np.float32)[rows].T.astype(np.float16))
        emb0 = np.ascontiguousarray(
            (np.asarray(W_ih0, np.float32)[rows].T + b0[rows][None, :])
            .astype(np.float16))
        bias0 = np.ascontiguousarray(np.broadcast_to(b0[rows], (B, G)).astype(np.float32))
        bias1 = np.ascontiguousarray(np.broadcast_to(b1[rows], (B, G)).astype(np.float32))
        woutt = np.ascontiguousarray(np.asarray(W_out, np.float32)[u].T.astype(np.float16))
        in_maps.append({
            "audio": np.ascontiguousarray(audio[8 * k: 8 * (k + 1)]),
            "tidx": tidx,
            "wproj": wproj_t,
            "bproj": bproj,
            "wh0t": wh0t, "wi0t": wi0t, "wi1t": wi1t, "wh1t": wh1t,
            "emb0": emb0, "bias0": bias0, "bias1": bias1,
            "woutt": woutt, "bout": np.ascontiguousarray(bout_f[u.start:u.stop]),
        })
    return in_maps


class _SpmdRunner:
    """Builds the sharded jit once; warm calls avoid re-tracing."""

    def __init__(self, nc, n_cores):
        import jax
        from jax.sharding import Mesh, PartitionSpec
        from jax.experimental.shard_map import shard_map
        from concourse.bass2jax import (_bass_exec_p, install_neuronx_cc_hook,
                                        partition_id_tensor)
        self._jax = jax
        install_neuronx_cc_hook()
        self.nc = nc
        self.n_cores = n_cores
        partition_name = (nc.partition_id_tensor.name
                          if nc.partition_id_tensor else None)
        in_names, out_names, out_avals, zero_outs = [], [], [], []
        for alloc in nc.m.functions[0].allocations:
            if not isinstance(alloc, mybir.MemoryLocationSet):
                continue
            name = alloc.memorylocations[0].name
            if alloc.kind == "ExternalInput":
                if name != partition_name:
                    in_names.append(name)
            elif alloc.kind == "ExternalOutput":
                out_names.append(name)
                shape = tuple(alloc.tensor_shape)
                dtype = mybir.dt.np(alloc.dtype)
                out_avals.append(jax.core.ShapedArray(shape, dtype))
                zero_outs.append(np.zeros(shape, dtype))
        self.in_names, self.out_names = in_names, out_names
        self.out_avals, self.zero_outs = out_avals, zero_outs
        n_params, n_outs = len(in_names), len(out_avals)
        all_in = list(in_names) + list(out_names)
        if partition_name is not None:
            all_in.append(partition_name)

        def _body(*args):
            operands = list(args)
            if partition_name is not None:
                operands.append(partition_id_tensor())
            outs = _bass_exec_p.bind(
                *operands, out_avals=tuple(out_avals), in_names=tuple(all_in),
                out_names=tuple(out_names), lowering_input_output_aliases=(),
                sim_require_finite=True, sim_require_nnan=True, nc=nc,
            )
            return tuple(outs)

        devices = jax.devices()[:n_cores]
        mesh = Mesh(np.asarray(devices), ("core",))
        in_specs = (PartitionSpec("core"),) * (n_params + n_outs)
        out_specs = (PartitionSpec("core"),) * n_outs
        self._fn = jax.jit(
            shard_map(_body, mesh=mesh, in_specs=in_specs,
                      out_specs=out_specs, check_rep=False),
            keep_unused=True,
        )
        self._zeros_dev = [
            jax.device_put(np.zeros((n_cores * z.shape[0], *z.shape[1:]), z.dtype))
            for z in zero_outs
        ]

    def prep_inputs(self, in_maps):
        return [
            self._jax.device_put(np.concatenate(
                [np.asarray(in_maps[c][n]) for c in range(self.n_cores)], axis=0))
            for n in self.in_names
        ]

    def run(self, concat_in):
        out = self._fn(*concat_in, *self._zeros_dev)
        self._jax.block_until_ready(out)
        return out

    def results(self, out_arrs):
        return [
            {n: np.asarray(out_arrs[i]).reshape(self.n_cores,
                                                *self.out_avals[i].shape)[c]
             for i, n in enumerate(self.out_names)}
            for c in range(self.n_cores)
        ]


_CACHE = {}


def get_runner(n_super=N_SUPER, dbg=False, **kw):
    key = (n_super, dbg, tuple(sorted(kw.items())))
    if key not in _CACHE:
        nc = bacc.Bacc("TRN2", target_bir_lowering=False, debug=False,
                       num_devices=N_CORES)
        _emit(nc, n_super, dbg=dbg, **kw)
        _CACHE[key] = _SpmdRunner(nc, N_CORES)
    return _CACHE[key]


def assemble(results, n_super=N_SUPER):
    steps = n_super - 2
    full = np.empty((B, steps, H), np.float32)
    for k in range(N_CORES):
        # per-core logits: [steps, U, B] -> [B, steps, U]
        full[:, :, 128 * k:128 * (k + 1)] = np.transpose(
            results[k]["logits"], (2, 0, 1))
    return full


def kernel(**inputs):
    in_maps = _prep_in_maps(**inputs)
    runner = get_runner()
    out = runner.run(runner.prep_inputs(in_maps))
    return assemble(runner.results(out))


# revision 25
# speedup vs baseline: 1.4987x; 1.4987x over previous
"""ASR decoder (2-layer LSTM, H=1024, B=64, 127 steps) on 8 Trainium2 cores.

Strategy: gate-sharding. Each core owns 128 of the 1024 hidden units of each
LSTM layer (i.e. 512 of the 4096 gate rows), with the full batch of 64.
Per "superstep" s the wavefront computes, fully in parallel per core:
  - L0: h0[t=s]    = LSTM0(x_s, h0[s-1])        (8 fp16 matmuls + emb gather)
  - L1: h1[t=s-1]  = LSTM1(h0[s-1], h1[s-2])    (16 fp16 matmuls)
  - logits[t=s-2]  = W_out_shard @ h1[s-2]      (8 fp16 matmuls)
then one 8-core AllGather exchanges the two fresh 128-unit h-chunks
(fp16, [256,64] per rank) so every core has the full h vectors next step.

Performance notes (measured on the axon-tunneled trn2.8x1):
  - compute+DMA per superstep is ~6us (cost model and HW agree); the
    per-step AllGather adds ~18us of critical-path latency (the recurrence
    cycle L0 -> AG -> L0 cannot hide it), so the kernel runs at the
    collective-latency floor: ~2.5-3ms total vs ~9ms+ for the data-parallel
    alternative (which re-streams all 12.6M weights through the PE every
    step on every core).
  - remote_dma/remote_sem_update broadcasts (the cheaper SBUF-to-SBUF
    exchange) hang in this PJRT environment and cannot be used.

Algebraic simplifications vs the reference:
  - mean-pool commutes with the linear projection: project mean(audio) only.
  - the one-hot @ W_ih0 matmul is an embedding row-gather (indirect DMA).
dtypes: fp16 weights/activations on the PE (1 cycle/row vs 4 for fp32),
fp32 PSUM accumulation, fp32 cell state and gate activations.
"""
import numpy as np

import concourse.bacc as bacc
import concourse.bass as bass
import concourse.mybir as mybir
import concourse.tile as tile
from concourse.masks import make_identity

F32 = mybir.dt.float32
F16 = mybir.dt.float16
I32 = mybir.dt.int32
AF = mybir.ActivationFunctionType

N_CORES = 8
B = 64            # batch
T_A = 500         # audio time
D_IN = 768        # audio dim
H = 1024          # hidden
G = 512           # gate rows per core (4 gates x 128 units)
U = 128           # units per core
SEQ = 128
STEPS = SEQ - 1   # 127 LSTM steps / output positions
N_SUPER = STEPS + 2  # wavefront supersteps


def _emit(nc, n_super, dbg=False, no_cc=False, no_emb=False, no_logits=False, repeat=1, split_ag=False):
    """Emit the whole kernel body under a TileContext."""
    steps = n_super - 2  # number of time steps actually computed

    # ---------------- DRAM I/O (per core) ----------------
    KC = H // 128  # 8 contraction chunks
    audio = nc.dram_tensor("audio", [B // N_CORES, T_A, D_IN], F32, kind="ExternalInput")
    tidx = nc.dram_tensor("tidx", [B, SEQ], I32, kind="ExternalInput")
    embx = nc.dram_tensor("embx", [max(steps - 1, 1), B, G], F16,
                          kind="ExternalInput")  # pre-gathered emb rows, t=1..steps-1
    wproj = nc.dram_tensor("wproj", [D_IN, H], F16, kind="ExternalInput")     # (W_proj/500).T
    bproj = nc.dram_tensor("bproj", [H], F32, kind="ExternalInput")
    wh0t = nc.dram_tensor("wh0t", [H, G], F16, kind="ExternalInput")          # W_hh0[rows_k].T
    wi0t = nc.dram_tensor("wi0t", [H, G], F16, kind="ExternalInput")          # W_ih0[rows_k].T
    wi1t = nc.dram_tensor("wi1t", [H, G], F16, kind="ExternalInput")          # W_ih1[rows_k].T
    wh1t = nc.dram_tensor("wh1t", [H, G], F16, kind="ExternalInput")          # W_hh1[rows_k].T
    emb0 = nc.dram_tensor("emb0", [H, G], F16, kind="ExternalInput")          # W_ih0[rows_k].T + b0
    bias0 = nc.dram_tensor("bias0", [B, G], F32, kind="ExternalInput")        # b0 broadcast
    bias1 = nc.dram_tensor("bias1", [B, G], F32, kind="ExternalInput")        # b1 broadcast
    woutt = nc.dram_tensor("woutt", [H, U], F16, kind="ExternalInput")        # W_out[rows char].T
    bout = nc.dram_tensor("bout", [U], F32, kind="ExternalInput")
    logits = nc.dram_tensor("logits", [steps, U, B], F32, kind="ExternalOutput")
    if dbg:
        d_mean = nc.dram_tensor("d_mean", [1, B // N_CORES * D_IN], F16, kind="ExternalOutput")
        d_enc = nc.dram_tensor("d_enc", [128, KC, B // N_CORES], F16, kind="ExternalOutput")
        d_x0 = nc.dram_tensor("d_x0", [128, KC, B], F16, kind="ExternalOutput")
        d_h0c0 = nc.dram_tensor("d_h0c0", [128, B], F16, kind="ExternalOutput")
        d_h0s0 = nc.dram_tensor("d_h0s0", [128, KC, B], F16, kind="ExternalOutput")
        d_h1s1 = nc.dram_tensor("d_h1s1", [128, KC, B], F16, kind="ExternalOutput")
        d_emb1 = nc.dram_tensor("d_emb1", [B, G], F16, kind="ExternalOutput")
        d_g0s1 = nc.dram_tensor("d_g0s1", [B, G], F32, kind="ExternalOutput")

    with tile.TileContext(nc) as tc:
        with (
            tc.tile_pool(name="wpool", bufs=1) as wpool,
            tc.tile_pool(name="state", bufs=1) as state,
            tc.tile_pool(name="dram", bufs=1, space="DRAM") as dpool,
            tc.tile_pool(name="hpool", bufs=2) as hpool,
            tc.tile_pool(name="gpool", bufs=3) as gpool,
            tc.tile_pool(name="psg", bufs=2, space="PSUM") as psg,
            tc.tile_pool(name="pst", bufs=2, space="PSUM") as pst,
            tc.tile_pool(name="psl", bufs=2, space="PSUM") as psl,
        ):
            # ---------------- persistent SBUF ----------------
            wh0_sb = wpool.tile([128, KC, G], F16, name="wh0_sb")
            wi1_sb = wpool.tile([128, KC, G], F16, name="wi1_sb")
            wh1_sb = wpool.tile([128, KC, G], F16, name="wh1_sb")
            wout_sb = wpool.tile([128, KC, U], F16, name="wout_sb")
            bias0_sb = wpool.tile([B, G], F32, name="bias0_sb")
            bias1_sb = wpool.tile([B, G], F32, name="bias1_sb")
            bout_sb = wpool.tile([U, 1], F32, name="bout_sb")
            idx_sb = wpool.tile([B, SEQ], I32, name="idx_sb")
            ident = wpool.tile([128, 128], F16, name="ident")
            c0_sb = state.tile([B, U], F32, name="c0_sb")
            c1_sb = state.tile([B, U], F32, name="c1_sb")

            nc.sync.dma_start(wh0_sb[:], wh0t.ap().rearrange("(c p) g -> p c g", p=128))
            nc.sync.dma_start(wi1_sb[:], wi1t.ap().rearrange("(c p) g -> p c g", p=128))
            nc.sync.dma_start(wh1_sb[:], wh1t.ap().rearrange("(c p) g -> p c g", p=128))
            nc.sync.dma_start(wout_sb[:], woutt.ap().rearrange("(c p) u -> p c u", p=128))
            nc.sync.dma_start(bias0_sb[:], bias0.ap())
            nc.sync.dma_start(bias1_sb[:], bias1.ap())
            nc.sync.dma_start(bout_sb[:], bout.ap().rearrange("(u one) -> u one", one=1))
            nc.sync.dma_start(idx_sb[:], tidx.ap())
            make_identity(nc, ident[:])
            nc.gpsimd.memset(c0_sb[:], 0.0)
            nc.gpsimd.memset(c1_sb[:], 0.0)

            # collective buffers
            cc_in = dpool.tile([2 * 128, B], F16, name="cc_in", bufs=2)
            cc_out = dpool.tile([N_CORES * 2 * 128, B], F16, name="cc_out",
                                addr_space="Shared", bufs=2)

            # =============== prologue: audio mean + projection ===============
            with (
                tc.tile_pool(name="apool", bufs=2) as apool,
                tc.tile_pool(name="appsum", bufs=1, space="PSUM") as appsum,
                tc.tile_pool(name="prpool", bufs=1) as prpool,
            ):
                ones_sb = prpool.tile([128, 1], F16, name="ones_sb")
                nc.gpsimd.memset(ones_sb[:], 1.0)
                wproj_sb = prpool.tile([128, 6, H], F16, name="wproj_sb")
                nc.sync.dma_start(wproj_sb[:], wproj.ap().rearrange("(c p) h -> p c h", p=128))
                bproj_sb = prpool.tile([128, KC], F32, name="bproj_sb")
                nc.sync.dma_start(bproj_sb[:], bproj.ap().rearrange("(c p) -> p c", p=128))

                # audio sum over time: 4 chunks of <=128 time rows
                a_t = audio.ap().rearrange("b t d -> t b d")
                tchunks = [(0, 128), (128, 128), (256, 128), (384, 116)]
                a16s = []
                for (t0, tcnt) in tchunks:
                    a32 = apool.tile([128, B // N_CORES, D_IN], F32, name="a32")
                    nc.sync.dma_start(a32[:tcnt], a_t[t0:t0 + tcnt])
                    a16 = gpool.tile([128, B // N_CORES * D_IN], F16, name="a16",
                                     tag="a16", bufs=4)
                    nc.scalar.activation(a16[:tcnt], a32[:tcnt].rearrange("p b d -> p (b d)"),
                                         AF.Copy)
                    a16s.append((a16, tcnt))
                # ones-matmul reduce: psum [1, 1024] per group (2 banks)
                mean16 = prpool.tile([1, B // N_CORES * D_IN], F16, name="mean16")
                for grp in range(6):
                    ps_m = appsum.tile([1, 1024], F32, name="ps_m", tag="ps_m")
                    for nn in range(2):
                        o = grp * 1024 + nn * 512
                        for ti, (a16, tcnt) in enumerate(a16s):
                            nc.tensor.matmul(
                                ps_m[:, nn * 512:(nn + 1) * 512],
                                ones_sb[:tcnt, :],
                                a16[:tcnt, o: o + 512],
                                start=(ti == 0), stop=(ti == 3),
                            )
                    nc.scalar.activation(mean16[:, grp * 1024:(grp + 1) * 1024],
                                         ps_m[:], AF.Copy)
                # redistribute [1, 8*768] -> [8, 768] via DRAM scratch
                mean_dr = dpool.tile([B // N_CORES, D_IN], F16, name="mean_dr")
                nc.sync.dma_start(mean_dr[:].rearrange("b d -> (b d)")[None, :], mean16[:])
                mean8 = prpool.tile([B // N_CORES, D_IN], F16, name="mean8")
                nc.sync.dma_start(mean8[:], mean_dr[:])
                # transpose to [d, b]: 6 PE transposes of [8, 128]
                meanT = prpool.tile([128, 6, B // N_CORES], F16, name="meanT")
                for dc in range(6):
                    ps_t = pst.tile([128, B], F16, name="ps_t", tag="t")
                    nc.tensor.transpose(ps_t[:, :8], mean8[:, dc * 128:(dc + 1) * 128],
                                        ident[:8, :8])
                    nc.vector.tensor_copy(meanT[:, dc, :], ps_t[:, :8])
                # projection: enc_k [h, 8] = W_proj' @ mean_k + b_proj
                enc_sb = prpool.tile([128, KC, B // N_CORES], F16, name="enc_sb")
                for hc in range(KC):
                    ps_p = psl.tile([128, B], F32, name="ps_p", tag="l")
                    for dc in range(6):
                        nc.tensor.matmul(ps_p[:, :8],
                                         wproj_sb[:, dc, hc * 128:(hc + 1) * 128],
                                         meanT[:, dc, :],
                                         start=(dc == 0), stop=(dc == 5))
                    nc.scalar.activation(enc_sb[:, hc, :], ps_p[:, :8], AF.Identity,
                                         bias=bproj_sb[:, hc:hc + 1])
                enc_dram = dpool.tile([H, B // N_CORES], F16, name="enc_dram")
                nc.sync.dma_start(enc_dram[:].rearrange("(c p) b -> p c b", p=128), enc_sb[:])
                x0_sb = prpool.tile([128, KC, B], F16, name="x0_sb")
                if no_cc:
                    nc.gpsimd.memset(x0_sb[:], 0.01)
                else:
                    enc_all = dpool.tile([N_CORES * H, B // N_CORES], F16, name="enc_all",
                                         addr_space="Shared")
                    nc.gpsimd.collective_compute(
                        "AllGather", mybir.AluOpType.bypass,
                        replica_groups=[list(range(N_CORES))],
                        ins=[enc_dram.opt()], outs=[enc_all.opt()],
                    )
                    # readback x0 stationary chunks [128, hc, B]
                    ea = enc_all[:].rearrange("(r c p) b -> c p r b", r=N_CORES, p=128)
                    for hc in range(KC):
                        nc.sync.dma_start(
                            x0_sb[:, hc, :].rearrange("p (r b) -> p r b", r=N_CORES),
                            ea[hc])
                # t=0 input weights
                wi0_sb = prpool.tile([128, KC, G], F16, name="wi0_sb")
                nc.sync.dma_start(wi0_sb[:], wi0t.ap().rearrange("(c p) g -> p c g", p=128))

                if dbg:
                    nc.sync.dma_start(d_mean.ap(), mean16[:])
                    nc.sync.dma_start(d_enc.ap(), enc_sb[:])
                    nc.sync.dma_start(d_x0.ap(), x0_sb[:])
                # zero tile for the h1 slot of the first AllGather
                zero16 = prpool.tile([128, B], F16, name="zero16")
                nc.gpsimd.memset(zero16[:], 0.0)

                # =============== main wavefront loop ===============
                h0_stat = None
                h1_stat = None
                for rep, s in [(rp, sp) for rp in range(repeat)
                               for sp in range(n_super)]:
                    # ---- A) layer 0, t = s ----
                    if s <= steps - 1:
                        ps_g0 = psg.tile([B, G], F32, name="ps_g0", tag="g")
                        if s == 0:
                            for c in range(KC):
                                nc.tensor.matmul(ps_g0[:], x0_sb[:, c, :],
                                                 wi0_sb[:, c, :],
                                                 start=(c == 0), stop=(c == KC - 1))
                        else:
                            for c in range(KC):
                                nc.tensor.matmul(ps_g0[:], h0_stat[:, c, :],
                                                 wh0_sb[:, c, :],
                                                 start=(c == 0), stop=(c == KC - 1))
                        g0_sb = gpool.tile([B, G], F32, name="g0_sb", tag="g0")
                        if s == 0 or no_emb:
                            nc.vector.tensor_add(g0_sb[:], ps_g0[:], bias0_sb[:])
                        else:
                            embt = gpool.tile([B, G], F16, name="embt", tag="emb")
                            nc.sync.dma_start(embt[:], embx[s - 1])
                            nc.vector.tensor_add(g0_sb[:], ps_g0[:], embt[:])
                            if dbg and s == 1:
                                nc.sync.dma_start(d_emb1.ap(), embt[:])
                                nc.sync.dma_start(d_g0s1.ap(), g0_sb[:])
                        # activations: cols [i(128) f(128) o(128) g(128)]
                        a0 = gpool.tile([B, G], F32, name="a0", tag="a0")
                        nc.scalar.activation(a0[:, 0:384], g0_sb[:, 0:384], AF.Sigmoid)
                        nc.scalar.activation(a0[:, 384:512], g0_sb[:, 384:512], AF.Tanh)
                        t1 = gpool.tile([B, U], F32, name="t1", tag="t1")
                        t2 = gpool.tile([B, U], F32, name="t2", tag="t2")
                        nc.vector.tensor_mul(t1[:], a0[:, 0:128], a0[:, 384:512])
                        nc.vector.tensor_mul(t2[:], a0[:, 128:256], c0_sb[:])
                        nc.vector.tensor_add(c0_sb[:], t1[:], t2[:])
                        tc0 = gpool.tile([B, U], F32, name="tc0", tag="tc0")
                        nc.scalar.activation(tc0[:], c0_sb[:], AF.Tanh)
                        h0T = gpool.tile([B, U], F16, name="h0T", tag="h0T")
                        nc.vector.tensor_mul(h0T[:], a0[:, 256:384], tc0[:])
                        ps_t0 = pst.tile([128, B], F16, name="ps_t0", tag="t")
                        nc.tensor.transpose(ps_t0[:, :B], h0T[:], ident[:B, :B])
                        h0_chunk = gpool.tile([128, B], F16, name="h0_chunk", tag="h0c")
                        nc.vector.tensor_copy(h0_chunk[:], ps_t0[:, :B])
                        if dbg and s == 0:
                            nc.sync.dma_start(d_h0c0.ap(), h0_chunk[:])

                    # ---- B) layer 1, t = s-1 ----
                    if 1 <= s <= steps:
                        ps_g1 = psg.tile([B, G], F32, name="ps_g1", tag="g")
                        for c in range(KC):
                            nc.tensor.matmul(ps_g1[:], h0_stat[:, c, :],
                                             wi1_sb[:, c, :],
                                             start=(c == 0),
                                             stop=(c == KC - 1 and s == 1))
                        if s >= 2:
                            for c in range(KC):
                                nc.tensor.matmul(ps_g1[:], h1_stat[:, c, :],
                                                 wh1_sb[:, c, :],
                                                 start=False, stop=(c == KC - 1))
                        g1_sb = gpool.tile([B, G], F32, name="g1_sb", tag="g1")
                        nc.vector.tensor_add(g1_sb[:], ps_g1[:], bias1_sb[:])
                        a1 = gpool.tile([B, G], F32, name="a1", tag="a1")
                        nc.scalar.activation(a1[:, 0:384], g1_sb[:, 0:384], AF.Sigmoid)
                        nc.scalar.activation(a1[:, 384:512], g1_sb[:, 384:512], AF.Tanh)
                        t3 = gpool.tile([B, U], F32, name="t3", tag="t3")
                        t4 = gpool.tile([B, U], F32, name="t4", tag="t4")
                        nc.vector.tensor_mul(t3[:], a1[:, 0:128], a1[:, 384:512])
                        nc.vector.tensor_mul(t4[:], a1[:, 128:256], c1_sb[:])
                        nc.vector.tensor_add(c1_sb[:], t3[:], t4[:])
                        tc1 = gpool.tile([B, U], F32, name="tc1", tag="tc1")
                        nc.scalar.activation(tc1[:], c1_sb[:], AF.Tanh)
                        h1T = gpool.tile([B, U], F16, name="h1T", tag="h1T")
                        nc.vector.tensor_mul(h1T[:], a1[:, 256:384], tc1[:])
                        ps_t1 = pst.tile([128, B], F16, name="ps_t1", tag="t")
                        nc.tensor.transpose(ps_t1[:, :B], h1T[:], ident[:B, :B])
                        h1_chunk = gpool.tile([128, B], F16, name="h1_chunk", tag="h1c")
                        nc.vector.tensor_copy(h1_chunk[:], ps_t1[:, :B])

                    # ---- C) logits, t = s-2 ----
                    if 2 <= s <= steps + 1 and not no_logits:
                        ps_lg = psl.tile([128, B], F32, name="ps_lg", tag="l")
                        for c in range(KC):
                            nc.tensor.matmul(ps_lg[:, :B], wout_sb[:, c, :],
                                             h1_stat[:, c, :],
                                             start=(c == 0), stop=(c == KC - 1))
                        lg_sb = gpool.tile([U, B], F32, name="lg_sb", tag="lg")
                        nc.scalar.activation(lg_sb[:], ps_lg[:, :B], AF.Identity,
                                             bias=bout_sb[:, 0:1])
                        nc.sync.dma_start(logits[s - 2], lg_sb[:])

                    # ---- E/F split: per-layer AllGather + readback ----
                    if split_ag:
                        if s <= steps - 1:
                            cc0_in = dpool.tile([128, B], F16, name="cc0_in",
                                                tag="cc0_in", bufs=2)
                            nc.sync.dma_start(cc0_in[:], h0_chunk[:])
                            cc0_out = dpool.tile([N_CORES * 128, B], F16,
                                                 name="cc0_out", tag="cc0_out",
                                                 addr_space="Shared", bufs=2)
                            if not no_cc:
                                nc.gpsimd.collective_compute(
                                    "AllGather", mybir.AluOpType.bypass,
                                    replica_groups=[list(range(N_CORES))],
                                    ins=[cc0_in.opt()], outs=[cc0_out.opt()],
                                )
                            co0 = cc0_out[:].rearrange("(r p) b -> p r b", r=N_CORES)
                            h0_new = hpool.tile([128, KC, B], F16, name="h0_new",
                                                tag="h0s")
                            for q in range(4):
                                nc.sync.dma_start(h0_new[:, 2 * q:2 * q + 2, :],
                                                  co0[:, 2 * q:2 * q + 2, :])
                            h0_stat = h0_new
                        if 1 <= s <= steps:
                            cc1_in = dpool.tile([128, B], F16, name="cc1_in",
                                                tag="cc1_in", bufs=2)
                            nc.sync.dma_start(cc1_in[:], h1_chunk[:])
                            cc1_out = dpool.tile([N_CORES * 128, B], F16,
                                                 name="cc1_out", tag="cc1_out",
                                                 addr_space="Shared", bufs=2)
                            if not no_cc:
                                nc.gpsimd.collective_compute(
                                    "AllGather", mybir.AluOpType.bypass,
                                    replica_groups=[list(range(N_CORES))],
                                    ins=[cc1_in.opt()], outs=[cc1_out.opt()],
                                )
                            co1 = cc1_out[:].rearrange("(r p) b -> p r b", r=N_CORES)
                            h1_new = hpool.tile([128, KC, B], F16, name="h1_new",
                                                tag="h1s")
                            for q in range(4):
                                nc.sync.dma_start(h1_new[:, 2 * q:2 * q + 2, :],
                                                  co1[:, 2 * q:2 * q + 2, :])
                            h1_stat = h1_new
                        continue
                    if s <= steps:
                        cc_in_s = dpool.tile([2 * 128, B], F16, name="cc_in",
                                             tag="cc_in", bufs=2)
                        if s <= steps - 1:
                            nc.sync.dma_start(cc_in_s[0:128], h0_chunk[:])
                        else:
                            nc.sync.dma_start(cc_in_s[0:128], zero16[:])
                        if s == 0:
                            nc.sync.dma_start(cc_in_s[128:256], zero16[:])
                        elif s >= 1:
                            nc.sync.dma_start(cc_in_s[128:256], h1_chunk[:])
                        cc_out_s = dpool.tile([N_CORES * 2 * 128, B], F16,
                                              name="cc_out", tag="cc_out",
                                              addr_space="Shared", bufs=2)
                        if not no_cc:
                            nc.gpsimd.collective_compute(
                                "AllGather", mybir.AluOpType.bypass,
                                replica_groups=[list(range(N_CORES))],
                                ins=[cc_in_s.opt()], outs=[cc_out_s.opt()],
                            )
                        # ---- F) readback ----
                        co = cc_out_s[:].rearrange("(r l p) b -> l p r b",
                                                     r=N_CORES, l=2, p=128)
                        h0_new = hpool.tile([128, KC, B], F16, name="h0_new", tag="h0s")
                        h1_new = hpool.tile([128, KC, B], F16, name="h1_new", tag="h1s")
                        if s <= steps - 1:
                            for q in range(4):
                                nc.sync.dma_start(h0_new[:, 2 * q:2 * q + 2, :],
                                                  co[0][:, 2 * q:2 * q + 2, :])
                        if s >= 1:
                            for q in range(4):
                                nc.sync.dma_start(h1_new[:, 2 * q:2 * q + 2, :],
                                                  co[1][:, 2 * q:2 * q + 2, :])
                        if dbg and s == 0:
                            nc.sync.dma_start(d_h0s0.ap(), h0_new[:])
                        if dbg and s == 1:
                            nc.sync.dma_start(d_h1s1.ap(), h1_new[:])
                        h0_stat = h0_new
                        h1_stat = h1_new

    nc.compile()
    return nc


# ----------------------------------------------------------------------------
# host side
# ----------------------------------------------------------------------------
def _prep_in_maps(audio_features, text_indices, W_proj, b_proj,
                  W_ih0, W_hh0, b_ih0, b_hh0,
                  W_ih1, W_hh1, b_ih1, b_hh1,
                  W_out, b_out):
    audio = np.ascontiguousarray(np.asarray(audio_features, np.float32))
    tidx = np.ascontiguousarray(np.asarray(text_indices).astype(np.int32))
    wproj_t = np.ascontiguousarray((np.asarray(W_proj, np.float32) / T_A).T
                                   .astype(np.float16))
    bproj = np.asarray(b_proj, np.float32)
    b0 = np.asarray(b_ih0, np.float32) + np.asarray(b_hh0, np.float32)
    b1 = np.asarray(b_ih1, np.float32) + np.asarray(b_hh1, np.float32)
    bout_f = np.asarray(b_out, np.float32)

    in_maps = []
    for k in range(N_CORES):
        u = slice(128 * k, 128 * (k + 1))
        # gate column order [i | f | o | g]  (pytorch rows: i, f, g, o)
        rows = np.concatenate([
            np.arange(128 * k, 128 * (k + 1)),            # i
            H + np.arange(128 * k, 128 * (k + 1)),        # f
            3 * H + np.arange(128 * k, 128 * (k + 1)),    # o
            2 * H + np.arange(128 * k, 128 * (k + 1)),    # g
        ])
        wh0t = np.ascontiguousarray(np.asarray(W_hh0, np.float32)[rows].T.astype(np.float16))
        wi0t = np.ascontiguousarray(np.asarray(W_ih0, np.float32)[rows].T.astype(np.float16))
        wi1t = np.ascontiguousarray(np.asarray(W_ih1, np.float32)[rows].T.astype(np.float16))
        wh1t = np.ascontiguousarray(np.asarray(W_hh1, np.float32)[rows].T.astype(np.float16))
        emb0 = np.ascontiguousarray(
            (np.asarray(W_ih0, np.float32)[rows].T + b0[rows][None, :])
            .astype(np.float16))
        bias0 = np.ascontiguousarray(np.broadcast_to(b0[rows], (B, G)).astype(np.float32))
        bias1 = np.ascontiguousarray(np.broadcast_to(b1[rows], (B, G)).astype(np.float32))
        woutt = np.ascontiguousarray(np.asarray(W_out, np.float32)[u].T.astype(np.float16))
        embx = np.ascontiguousarray(
            emb0[tidx[:, 1:STEPS]].transpose(1, 0, 2))  # [STEPS-1, B, G]
        in_maps.append({
            "embx": embx,
            "audio": np.ascontiguousarray(audio[8 * k: 8 * (k + 1)]),
            "tidx": tidx,
            "wproj": wproj_t,
            "bproj": bproj,
            "wh0t": wh0t, "wi0t": wi0t, "wi1t": wi1t, "wh1t": wh1t,
            "emb0": emb0, "bias0": bias0, "bias1": bias1,
            "woutt": woutt, "bout": np.ascontiguousarray(bout_f[u.start:u.stop]),
        })
    return in_maps


class _SpmdRunner:
    """Builds the sharded jit once; warm calls avoid re-tracing."""

    def __init__(self, nc, n_cores):
        import jax
        from jax.sharding import Mesh, PartitionSpec
        from jax.experimental.shard_map import shard_map
        from concourse.bass2jax import (_bass_exec_p, install_neuronx_cc_hook,
                                        partition_id_tensor)
        self._jax = jax
        install_neuronx_cc_hook()
        self.nc = nc
        self.n_cores = n_cores
        partition_name = (nc.partition_id_tensor.name
                          if nc.partition_id_tensor else None)
        in_names, out_names, out_avals, zero_outs = [], [], [], []
        for alloc in nc.m.functions[0].allocations:
            if not isinstance(alloc, mybir.MemoryLocationSet):
                continue
            name = alloc.memorylocations[0].name
            if alloc.kind == "ExternalInput":
                if name != partition_name:
                    in_names.append(name)
            elif alloc.kind == "ExternalOutput":
                out_names.append(name)
                shape = tuple(alloc.tensor_shape)
                dtype = mybir.dt.np(alloc.dtype)
                out_avals.append(jax.core.ShapedArray(shape, dtype))
                zero_outs.append(np.zeros(shape, dtype))
        self.in_names, self.out_names = in_names, out_names
        self.out_avals, self.zero_outs = out_avals, zero_outs
        n_params, n_outs = len(in_names), len(out_avals)
        all_in = list(in_names) + list(out_names)
        if partition_name is not None:
            all_in.append(partition_name)

        def _body(*args):
            operands = list(args)
            if partition_name is not None:
                operands.append(partition_id_tensor())
            outs = _bass_exec_p.bind(
                *operands, out_avals=tuple(out_avals), in_names=tuple(all_in),
                out_names=tuple(out_names), lowering_input_output_aliases=(),
                sim_require_finite=True, sim_require_nnan=True, nc=nc,
            )
            return tuple(outs)

        devices = jax.devices()[:n_cores]
        mesh = Mesh(np.asarray(devices), ("core",))
        in_specs = (PartitionSpec("core"),) * (n_params + n_outs)
        out_specs = (PartitionSpec("core"),) * n_outs
        self._fn = jax.jit(
            shard_map(_body, mesh=mesh, in_specs=in_specs,
                      out_specs=out_specs, check_rep=False),
            keep_unused=True,
        )
        self._zeros_dev = [
            jax.device_put(np.zeros((n_cores * z.shape[0], *z.shape[1:]), z.dtype))
            for z in zero_outs
        ]

    def prep_inputs(self, in_maps):
        return [
            self._jax.device_put(np.concatenate(
                [np.asarray(in_maps[c][n]) for c in range(self.n_cores)], axis=0))
            for n in self.in_names
        ]

    def run(self, concat_in):
        out = self._fn(*concat_in, *self._zeros_dev)
        self._jax.block_until_ready(out)
        return out

    def results(self, out_arrs):
        return [
            {n: np.asarray(out_arrs[i]).reshape(self.n_cores,
                                                *self.out_avals[i].shape)[c]
             for i, n in enumerate(self.out_names)}
            for c in range(self.n_cores)
        ]


_CACHE = {}


def get_runner(n_super=N_SUPER, dbg=False, **kw):
    key = (n_super, dbg, tuple(sorted(kw.items())))
    if key not in _CACHE:
        nc = bacc.Bacc("TRN2", target_bir_lowering=False, debug=False,
                       num_devices=N_CORES)
        _emit(nc, n_super, dbg=dbg, **kw)
        _CACHE[key] = _SpmdRunner(nc, N_CORES)
    return _CACHE[key]


def assemble(results, n_super=N_SUPER):
    steps = n_super - 2
    full = np.empty((B, steps, H), np.float32)
    for k in range(N_CORES):
        # per-core logits: [steps, U, B] -> [B, steps, U]
        full[:, :, 128 * k:128 * (k + 1)] = np.transpose(
            results[k]["logits"], (2, 0, 1))
    return full


def kernel(**inputs):
    in_maps = _prep_in_maps(**inputs)
    runner = get_runner()
    out = runner.run(runner.prep_inputs(in_maps))
    return assemble(runner.results(out))


# revision 34
# speedup vs baseline: 1.7302x; 1.1544x over previous
"""ASR decoder (2-layer LSTM, H=1024, B=64, 127 steps) on 8 Trainium2 cores.

Strategy: gate-sharding. Each core owns 128 of the 1024 hidden units of each
LSTM layer (i.e. 512 of the 4096 gate rows), with the full batch of 64.
Per "superstep" s the wavefront computes, fully in parallel per core:
  - L0: h0[t=s]    = LSTM0(x_s, h0[s-1])        (8 fp16 matmuls + emb gather)
  - L1: h1[t=s-1]  = LSTM1(h0[s-1], h1[s-2])    (16 fp16 matmuls)
  - logits[t=s-2]  = W_out_shard @ h1[s-2]      (8 fp16 matmuls)
then one 8-core AllGather exchanges the two fresh 128-unit h-chunks
(fp16, [256,64] per rank) so every core has the full h vectors next step.

Performance notes (measured on the axon-tunneled trn2.8x1):
  - compute+DMA per superstep is ~6us (cost model and HW agree); the
    per-step AllGather adds ~12-18us of critical-path latency (the
    recurrence cycle L0 -> AG -> L0 cannot hide it), so the kernel runs
    near the collective-latency floor: ~2.5-3.2ms total vs ~9ms+ for the
    data-parallel alternative (which re-streams all 12.6M weights through
    the PE every step on every core).
  - the teacher-forcing embedding rows are pre-gathered on the host
    (zero-FLOP index selection) into the `embx` input: an on-device
    indirect-DMA gather per step serializes with the collective on the
    in-order gpsimd queue and cost ~38us/step (6.3ms vs 1.5ms loop time).
  - remote_dma/remote_sem_update broadcasts (the cheaper SBUF-to-SBUF
    exchange) hang in this PJRT environment and cannot be used; splitting
    the AllGather per layer or issuing DMAs on the Activation HWDGE queue
    both measured slower.

Algebraic simplifications vs the reference:
  - mean-pool commutes with the linear projection: project mean(audio) only.
  - the one-hot @ W_ih0 matmul is an embedding row-gather (indirect DMA).
dtypes: fp16 weights/activations on the PE (1 cycle/row vs 4 for fp32),
fp32 PSUM accumulation, fp32 cell state and gate activations.
"""
import numpy as np

import concourse.bacc as bacc
import concourse.bass as bass
import concourse.mybir as mybir
import concourse.tile as tile
from concourse.masks import make_identity

F32 = mybir.dt.float32
F16 = mybir.dt.float16
I32 = mybir.dt.int32
AF = mybir.ActivationFunctionType

N_CORES = 8
B = 64            # batch
T_A = 500         # audio time
D_IN = 768        # audio dim
H = 1024          # hidden
G = 512           # gate rows per core (4 gates x 128 units)
U = 128           # units per core
SEQ = 128
STEPS = SEQ - 1   # 127 LSTM steps / output positions
N_SUPER = STEPS + 2  # wavefront supersteps


def _emit(nc, n_super, dbg=False, no_cc=False, no_emb=False, no_logits=False, repeat=1, split_ag=False, boost=True):
    """Emit the whole kernel body under a TileContext."""
    steps = n_super - 2  # number of time steps actually computed

    # ---------------- DRAM I/O (per core) ----------------
    KC = H // 128  # 8 contraction chunks
    audio = nc.dram_tensor("audio", [B // N_CORES, T_A, D_IN], F32, kind="ExternalInput")
    tidx = nc.dram_tensor("tidx", [B, SEQ], I32, kind="ExternalInput")
    embx = nc.dram_tensor("embx", [max(steps - 1, 1), B, G], F16,
                          kind="ExternalInput")  # pre-gathered emb rows, t=1..steps-1
    wproj = nc.dram_tensor("wproj", [D_IN, H], F16, kind="ExternalInput")     # (W_proj/500).T
    bproj = nc.dram_tensor("bproj", [H], F32, kind="ExternalInput")
    wh0t = nc.dram_tensor("wh0t", [H, G], F16, kind="ExternalInput")          # W_hh0[rows_k].T
    wi0t = nc.dram_tensor("wi0t", [H, G], F16, kind="ExternalInput")          # W_ih0[rows_k].T
    wi1t = nc.dram_tensor("wi1t", [H, G], F16, kind="ExternalInput")          # W_ih1[rows_k].T
    wh1t = nc.dram_tensor("wh1t", [H, G], F16, kind="ExternalInput")          # W_hh1[rows_k].T
    emb0 = nc.dram_tensor("emb0", [H, G], F16, kind="ExternalInput")          # W_ih0[rows_k].T + b0
    bias0 = nc.dram_tensor("bias0", [B, G], F32, kind="ExternalInput")        # b0 broadcast
    bias1 = nc.dram_tensor("bias1", [B, G], F32, kind="ExternalInput")        # b1 broadcast
    woutt = nc.dram_tensor("woutt", [H, U], F16, kind="ExternalInput")        # W_out[rows char].T
    bout = nc.dram_tensor("bout", [U], F32, kind="ExternalInput")
    logits = nc.dram_tensor("logits", [steps, U, B], F32, kind="ExternalOutput")
    if dbg:
        d_mean = nc.dram_tensor("d_mean", [1, B // N_CORES * D_IN], F16, kind="ExternalOutput")
        d_enc = nc.dram_tensor("d_enc", [128, KC, B // N_CORES], F16, kind="ExternalOutput")
        d_x0 = nc.dram_tensor("d_x0", [128, KC, B], F16, kind="ExternalOutput")
        d_h0c0 = nc.dram_tensor("d_h0c0", [128, B], F16, kind="ExternalOutput")
        d_h0s0 = nc.dram_tensor("d_h0s0", [128, KC, B], F16, kind="ExternalOutput")
        d_h1s1 = nc.dram_tensor("d_h1s1", [128, KC, B], F16, kind="ExternalOutput")
        d_emb1 = nc.dram_tensor("d_emb1", [B, G], F16, kind="ExternalOutput")
        d_g0s1 = nc.dram_tensor("d_g0s1", [B, G], F32, kind="ExternalOutput")

    hb = 4 if boost == 2 else (3 if boost else 2)
    gb = 3
    pb = 2
    with tile.TileContext(nc) as tc:
        with (
            tc.tile_pool(name="wpool", bufs=1) as wpool,
            tc.tile_pool(name="state", bufs=1) as state,
            tc.tile_pool(name="dram", bufs=1, space="DRAM") as dpool,
            tc.tile_pool(name="hpool", bufs=hb) as hpool,
            tc.tile_pool(name="gpool", bufs=gb) as gpool,
            tc.tile_pool(name="psg", bufs=pb, space="PSUM") as psg,
            tc.tile_pool(name="pst", bufs=2, space="PSUM") as pst,
            tc.tile_pool(name="psl", bufs=2, space="PSUM") as psl,
        ):
            # ---------------- persistent SBUF ----------------
            wh0_sb = wpool.tile([128, KC, G], F16, name="wh0_sb")
            wi1_sb = wpool.tile([128, KC, G], F16, name="wi1_sb")
            wh1_sb = wpool.tile([128, KC, G], F16, name="wh1_sb")
            wout_sb = wpool.tile([128, KC, U], F16, name="wout_sb")
            bias0_sb = wpool.tile([B, G], F32, name="bias0_sb")
            bias1_sb = wpool.tile([B, G], F32, name="bias1_sb")
            bout_sb = wpool.tile([U, 1], F32, name="bout_sb")
            idx_sb = wpool.tile([B, SEQ], I32, name="idx_sb")
            ident = wpool.tile([128, 128], F16, name="ident")
            c0_sb = state.tile([B, U], F32, name="c0_sb")
            c1_sb = state.tile([B, U], F32, name="c1_sb")

            nc.sync.dma_start(wh0_sb[:], wh0t.ap().rearrange("(c p) g -> p c g", p=128))
            nc.sync.dma_start(wi1_sb[:], wi1t.ap().rearrange("(c p) g -> p c g", p=128))
            nc.sync.dma_start(wh1_sb[:], wh1t.ap().rearrange("(c p) g -> p c g", p=128))
            nc.sync.dma_start(wout_sb[:], woutt.ap().rearrange("(c p) u -> p c u", p=128))
            nc.sync.dma_start(bias0_sb[:], bias0.ap())
            nc.sync.dma_start(bias1_sb[:], bias1.ap())
            nc.sync.dma_start(bout_sb[:], bout.ap().rearrange("(u one) -> u one", one=1))
            nc.sync.dma_start(idx_sb[:], tidx.ap())
            make_identity(nc, ident[:])
            nc.gpsimd.memset(c0_sb[:], 0.0)
            nc.gpsimd.memset(c1_sb[:], 0.0)

            # =============== prologue: audio mean + projection ===============
            with (
                tc.tile_pool(name="apool", bufs=2) as apool,
                tc.tile_pool(name="appsum", bufs=1, space="PSUM") as appsum,
                tc.tile_pool(name="prpool", bufs=1) as prpool,
            ):
                ones_sb = prpool.tile([128, 1], F16, name="ones_sb")
                nc.gpsimd.memset(ones_sb[:], 1.0)
                wproj_sb = prpool.tile([128, 6, H], F16, name="wproj_sb")
                nc.sync.dma_start(wproj_sb[:], wproj.ap().rearrange("(c p) h -> p c h", p=128))
                bproj_sb = prpool.tile([128, KC], F32, name="bproj_sb")
                nc.sync.dma_start(bproj_sb[:], bproj.ap().rearrange("(c p) -> p c", p=128))

                # audio sum over time: 4 chunks of <=128 time rows
                a_t = audio.ap().rearrange("b t d -> t b d")
                tchunks = [(0, 128), (128, 128), (256, 128), (384, 116)]
                a16s = []
                for (t0, tcnt) in tchunks:
                    a32 = apool.tile([128, B // N_CORES, D_IN], F32, name="a32")
                    nc.sync.dma_start(a32[:tcnt], a_t[t0:t0 + tcnt])
                    a16 = gpool.tile([128, B // N_CORES * D_IN], F16, name="a16",
                                     tag="a16", bufs=4)
                    nc.scalar.activation(a16[:tcnt], a32[:tcnt].rearrange("p b d -> p (b d)"),
                                         AF.Copy)
                    a16s.append((a16, tcnt))
                # ones-matmul reduce: psum [1, 1024] per group (2 banks)
                mean16 = prpool.tile([1, B // N_CORES * D_IN], F16, name="mean16")
                for grp in range(6):
                    ps_m = appsum.tile([1, 1024], F32, name="ps_m", tag="ps_m")
                    for nn in range(2):
                        o = grp * 1024 + nn * 512
                        for ti, (a16, tcnt) in enumerate(a16s):
                            nc.tensor.matmul(
                                ps_m[:, nn * 512:(nn + 1) * 512],
                                ones_sb[:tcnt, :],
                                a16[:tcnt, o: o + 512],
                                start=(ti == 0), stop=(ti == 3),
                            )
                    nc.scalar.activation(mean16[:, grp * 1024:(grp + 1) * 1024],
                                         ps_m[:], AF.Copy)
                # redistribute [1, 8*768] -> [8, 768] via DRAM scratch
                mean_dr = dpool.tile([B // N_CORES, D_IN], F16, name="mean_dr")
                nc.sync.dma_start(mean_dr[:].rearrange("b d -> (b d)")[None, :], mean16[:])
                mean8 = prpool.tile([B // N_CORES, D_IN], F16, name="mean8")
                nc.sync.dma_start(mean8[:], mean_dr[:])
                # transpose to [d, b]: 6 PE transposes of [8, 128]
                meanT = prpool.tile([128, 6, B // N_CORES], F16, name="meanT")
                for dc in range(6):
                    ps_t = pst.tile([128, B], F16, name="ps_t", tag="t")
                    nc.tensor.transpose(ps_t[:, :8], mean8[:, dc * 128:(dc + 1) * 128],
                                        ident[:8, :8])
                    nc.vector.tensor_copy(meanT[:, dc, :], ps_t[:, :8])
                # projection: enc_k [h, 8] = W_proj' @ mean_k + b_proj
                enc_sb = prpool.tile([128, KC, B // N_CORES], F16, name="enc_sb")
                for hc in range(KC):
                    ps_p = psl.tile([128, B], F32, name="ps_p", tag="l")
                    for dc in range(6):
                        nc.tensor.matmul(ps_p[:, :8],
                                         wproj_sb[:, dc, hc * 128:(hc + 1) * 128],
                                         meanT[:, dc, :],
                                         start=(dc == 0), stop=(dc == 5))
                    nc.scalar.activation(enc_sb[:, hc, :], ps_p[:, :8], AF.Identity,
                                         bias=bproj_sb[:, hc:hc + 1])
                enc_dram = dpool.tile([H, B // N_CORES], F16, name="enc_dram")
                nc.sync.dma_start(enc_dram[:].rearrange("(c p) b -> p c b", p=128), enc_sb[:])
                x0_sb = prpool.tile([128, KC, B], F16, name="x0_sb")
                if no_cc:
                    nc.gpsimd.memset(x0_sb[:], 0.01)
                else:
                    enc_all = dpool.tile([N_CORES * H, B // N_CORES], F16, name="enc_all",
                                         addr_space="Shared")
                    nc.gpsimd.collective_compute(
                        "AllGather", mybir.AluOpType.bypass,
                        replica_groups=[list(range(N_CORES))],
                        ins=[enc_dram.opt()], outs=[enc_all.opt()],
                    )
                    # readback x0 stationary chunks [128, hc, B]
                    ea = enc_all[:].rearrange("(r c p) b -> c p r b", r=N_CORES, p=128)
                    for hc in range(KC):
                        nc.sync.dma_start(
                            x0_sb[:, hc, :].rearrange("p (r b) -> p r b", r=N_CORES),
                            ea[hc])
                # t=0 input weights
                wi0_sb = prpool.tile([128, KC, G], F16, name="wi0_sb")
                nc.sync.dma_start(wi0_sb[:], wi0t.ap().rearrange("(c p) g -> p c g", p=128))

                if dbg:
                    nc.sync.dma_start(d_mean.ap(), mean16[:])
                    nc.sync.dma_start(d_enc.ap(), enc_sb[:])
                    nc.sync.dma_start(d_x0.ap(), x0_sb[:])
                # zero tile for the h1 slot of the first AllGather
                zero16 = prpool.tile([128, B], F16, name="zero16")
                nc.gpsimd.memset(zero16[:], 0.0)

                # =============== main wavefront loop ===============
                h0_stat = None
                h1_stat = None
                embt_cur = None
                for rep, s in [(rp, sp) for rp in range(repeat)
                               for sp in range(n_super)]:
                    if s <= steps:
                        hc2 = gpool.tile([128, 2, B], F16, name="hc2",
                                         tag="hc2", bufs=2)
                        if s == steps:
                            nc.vector.tensor_copy(hc2[:, 0, :], zero16[:])
                        if s == 0:
                            nc.vector.tensor_copy(hc2[:, 1, :], zero16[:])
                    # ---- A) layer 0, t = s ----
                    if s <= steps - 1:
                        ps_g0 = psg.tile([B, G], F32, name="ps_g0", tag="g")
                        if s == 0:
                            for c in range(KC):
                                nc.tensor.matmul(ps_g0[:], x0_sb[:, c, :],
                                                 wi0_sb[:, c, :],
                                                 start=(c == 0), stop=(c == KC - 1))
                        else:
                            for c in range(KC):
                                nc.tensor.matmul(ps_g0[:], h0_stat[:, c, :],
                                                 wh0_sb[:, c, :],
                                                 start=(c == 0), stop=(c == KC - 1))
                        g0_sb = gpool.tile([B, G], F32, name="g0_sb", tag="g0")
                        if s == 0 or no_emb:
                            nc.vector.tensor_add(g0_sb[:], ps_g0[:], bias0_sb[:])
                        else:
                            nc.vector.tensor_add(g0_sb[:], ps_g0[:], embt_cur[:])
                            if dbg and s == 1:
                                nc.sync.dma_start(d_emb1.ap(), embt_cur[:])
                                nc.sync.dma_start(d_g0s1.ap(), g0_sb[:])
                        # activations: cols [i(128) f(128) o(128) g(128)]
                        a0 = gpool.tile([B, G], F32, name="a0", tag="a0")
                        nc.scalar.activation(a0[:, 0:384], g0_sb[:, 0:384], AF.Sigmoid)
                        nc.scalar.activation(a0[:, 384:512], g0_sb[:, 384:512], AF.Tanh)
                        t1 = gpool.tile([B, U], F32, name="t1", tag="t1")
                        t2 = gpool.tile([B, U], F32, name="t2", tag="t2")
                        nc.vector.tensor_mul(t1[:], a0[:, 0:128], a0[:, 384:512])
                        nc.vector.tensor_mul(t2[:], a0[:, 128:256], c0_sb[:])
                        nc.vector.tensor_add(c0_sb[:], t1[:], t2[:])
                        tc0 = gpool.tile([B, U], F32, name="tc0", tag="tc0")
                        nc.scalar.activation(tc0[:], c0_sb[:], AF.Tanh)
                        h0T = gpool.tile([B, U], F16, name="h0T", tag="h0T")
                        nc.vector.tensor_mul(h0T[:], a0[:, 256:384], tc0[:])
                        ps_t0 = pst.tile([128, B], F16, name="ps_t0", tag="t")
                        nc.tensor.transpose(ps_t0[:, :B], h0T[:], ident[:B, :B])
                        nc.vector.tensor_copy(hc2[:, 0, :], ps_t0[:, :B])
                        if dbg and s == 0:
                            nc.sync.dma_start(d_h0c0.ap(), hc2[:, 0, :])

                    # ---- B) layer 1, t = s-1 ----
                    if 1 <= s <= steps:
                        ps_g1 = psg.tile([B, G], F32, name="ps_g1", tag="g")
                        for c in range(KC):
                            nc.tensor.matmul(ps_g1[:], h0_stat[:, c, :],
                                             wi1_sb[:, c, :],
                                             start=(c == 0),
                                             stop=(c == KC - 1 and s == 1))
                        if s >= 2:
                            for c in range(KC):
                                nc.tensor.matmul(ps_g1[:], h1_stat[:, c, :],
                                                 wh1_sb[:, c, :],
                                                 start=False, stop=(c == KC - 1))
                        g1_sb = gpool.tile([B, G], F32, name="g1_sb", tag="g1")
                        nc.vector.tensor_add(g1_sb[:], ps_g1[:], bias1_sb[:])
                        a1 = gpool.tile([B, G], F32, name="a1", tag="a1")
                        nc.scalar.activation(a1[:, 0:384], g1_sb[:, 0:384], AF.Sigmoid)
                        nc.scalar.activation(a1[:, 384:512], g1_sb[:, 384:512], AF.Tanh)
                        t3 = gpool.tile([B, U], F32, name="t3", tag="t3")
                        t4 = gpool.tile([B, U], F32, name="t4", tag="t4")
                        nc.vector.tensor_mul(t3[:], a1[:, 0:128], a1[:, 384:512])
                        nc.vector.tensor_mul(t4[:], a1[:, 128:256], c1_sb[:])
                        nc.vector.tensor_add(c1_sb[:], t3[:], t4[:])
                        tc1 = gpool.tile([B, U], F32, name="tc1", tag="tc1")
                        nc.scalar.activation(tc1[:], c1_sb[:], AF.Tanh)
                        h1T = gpool.tile([B, U], F16, name="h1T", tag="h1T")
                        nc.vector.tensor_mul(h1T[:], a1[:, 256:384], tc1[:])
                        ps_t1 = pst.tile([128, B], F16, name="ps_t1", tag="t")
                        nc.tensor.transpose(ps_t1[:, :B], h1T[:], ident[:B, :B])
                        nc.vector.tensor_copy(hc2[:, 1, :], ps_t1[:, :B])

                    # ---- C) logits, t = s-2 ----
                    if 2 <= s <= steps + 1 and not no_logits:
                        ps_lg = psl.tile([128, B], F32, name="ps_lg", tag="l")
                        for c in range(KC):
                            nc.tensor.matmul(ps_lg[:, :B], wout_sb[:, c, :],
                                             h1_stat[:, c, :],
                                             start=(c == 0), stop=(c == KC - 1))
                        lg_sb = gpool.tile([U, B], F32, name="lg_sb", tag="lg")
                        nc.scalar.activation(lg_sb[:], ps_lg[:, :B], AF.Identity,
                                             bias=bout_sb[:, 0:1])
                        lg_pending = (lg_sb, s - 2)

                    # ---- E/F split: per-layer AllGather + readback ----
                    if split_ag:
                        if s <= steps - 1:
                            cc0_in = dpool.tile([128, B], F16, name="cc0_in",
                                                tag="cc0_in", bufs=2)
                            nc.sync.dma_start(cc0_in[:], hc2[:, 0, :])
                            cc0_out = dpool.tile([N_CORES * 128, B], F16,
                                                 name="cc0_out", tag="cc0_out",
                                                 addr_space="Shared", bufs=2)
                            if not no_cc:
                                nc.gpsimd.collective_compute(
                                    "AllGather", mybir.AluOpType.bypass,
                                    replica_groups=[list(range(N_CORES))],
                                    ins=[cc0_in.opt()], outs=[cc0_out.opt()],
                                )
                            co0 = cc0_out[:].rearrange("(r p) b -> p r b", r=N_CORES)
                            h0_new = hpool.tile([128, KC, B], F16, name="h0_new",
                                                tag="h0s")
                            for q in range(4):
                                nc.sync.dma_start(h0_new[:, 2 * q:2 * q + 2, :],
                                                  co0[:, 2 * q:2 * q + 2, :])
                            h0_stat = h0_new
                        if 1 <= s <= steps:
                            cc1_in = dpool.tile([128, B], F16, name="cc1_in",
                                                tag="cc1_in", bufs=2)
                            nc.sync.dma_start(cc1_in[:], hc2[:, 1, :])
                            cc1_out = dpool.tile([N_CORES * 128, B], F16,
                                                 name="cc1_out", tag="cc1_out",
                                                 addr_space="Shared", bufs=2)
                            if not no_cc:
                                nc.gpsimd.collective_compute(
                                    "AllGather", mybir.AluOpType.bypass,
                                    replica_groups=[list(range(N_CORES))],
                                    ins=[cc1_in.opt()], outs=[cc1_out.opt()],
                                )
                            co1 = cc1_out[:].rearrange("(r p) b -> p r b", r=N_CORES)
                            h1_new = hpool.tile([128, KC, B], F16, name="h1_new",
                                                tag="h1s")
                            for q in range(4):
                                nc.sync.dma_start(h1_new[:, 2 * q:2 * q + 2, :],
                                                  co1[:, 2 * q:2 * q + 2, :])
                            h1_stat = h1_new
                        continue
                    if s <= steps:
                        cc_in_s = dpool.tile([2 * 128, B], F16, name="cc_in",
                                             tag="cc_in", bufs=hb)
                        nc.sync.dma_start(
                            cc_in_s[:].rearrange("(l p) b -> p l b", l=2), hc2[:])
                        cc_out_s = dpool.tile([N_CORES * 2 * 128, B], F16,
                                              name="cc_out", tag="cc_out",
                                              addr_space="Shared", bufs=hb)
                        if not no_cc:
                            nc.gpsimd.collective_compute(
                                "AllGather", mybir.AluOpType.bypass,
                                replica_groups=[list(range(N_CORES))],
                                ins=[cc_in_s.opt()], outs=[cc_out_s.opt()],
                            )
                        # ---- F) readback ----
                        co = cc_out_s[:].rearrange("(r l p) b -> l p r b",
                                                     r=N_CORES, l=2, p=128)
                        h0_new = hpool.tile([128, KC, B], F16, name="h0_new", tag="h0s")
                        h1_new = hpool.tile([128, KC, B], F16, name="h1_new", tag="h1s")
                        if s <= steps - 1:
                            for q in range(4):
                                nc.sync.dma_start(h0_new[:, 2 * q:2 * q + 2, :],
                                                  co[0][:, 2 * q:2 * q + 2, :])
                        if s >= 1:
                            for q in range(4):
                                nc.sync.dma_start(h1_new[:, 2 * q:2 * q + 2, :],
                                                  co[1][:, 2 * q:2 * q + 2, :])
                        if dbg and s == 0:
                            nc.sync.dma_start(d_h0s0.ap(), h0_new[:])
                        if dbg and s == 1:
                            nc.sync.dma_start(d_h1s1.ap(), h1_new[:])
                        h0_stat = h0_new
                        h1_stat = h1_new
                    # ---- tail: non-critical DMAs behind the exchange ----
                    if 2 <= s <= steps + 1 and not no_logits:
                        nc.sync.dma_start(logits[lg_pending[1]], lg_pending[0][:])
                    if 1 <= s + 1 <= steps - 1 and not no_emb:
                        embt_cur = gpool.tile([B, G], F16, name="embt", tag="emb",
                                              bufs=3)
                        nc.sync.dma_start(embt_cur[:], embx[s])

    nc.compile()
    return nc


# ----------------------------------------------------------------------------
# host side
# ----------------------------------------------------------------------------
def _prep_in_maps(audio_features, text_indices, W_proj, b_proj,
                  W_ih0, W_hh0, b_ih0, b_hh0,
                  W_ih1, W_hh1, b_ih1, b_hh1,
                  W_out, b_out):
    audio = np.ascontiguousarray(np.asarray(audio_features, np.float32))
    tidx = np.ascontiguousarray(np.asarray(text_indices).astype(np.int32))
    wproj_t = np.ascontiguousarray((np.asarray(W_proj, np.float32) / T_A).T
                                   .astype(np.float16))
    bproj = np.asarray(b_proj, np.float32)
    b0 = np.asarray(b_ih0, np.float32) + np.asarray(b_hh0, np.float32)
    b1 = np.asarray(b_ih1, np.float32) + np.asarray(b_hh1, np.float32)
    bout_f = np.asarray(b_out, np.float32)

    in_maps = []
    for k in range(N_CORES):
        u = slice(128 * k, 128 * (k + 1))
        # gate column order [i | f | o | g]  (pytorch rows: i, f, g, o)
        rows = np.concatenate([
            np.arange(128 * k, 128 * (k + 1)),            # i
            H + np.arange(128 * k, 128 * (k + 1)),        # f
            3 * H + np.arange(128 * k, 128 * (k + 1)),    # o
            2 * H + np.arange(128 * k, 128 * (k + 1)),    # g
        ])
        wh0t = np.ascontiguousarray(np.asarray(W_hh0, np.float32)[rows].T.astype(np.float16))
        wi0t = np.ascontiguousarray(np.asarray(W_ih0, np.float32)[rows].T.astype(np.float16))
        wi1t = np.ascontiguousarray(np.asarray(W_ih1, np.float32)[rows].T.astype(np.float16))
        wh1t = np.ascontiguousarray(np.asarray(W_hh1, np.float32)[rows].T.astype(np.float16))
        emb0 = np.ascontiguousarray(
            (np.asarray(W_ih0, np.float32)[rows].T + b0[rows][None, :])
            .astype(np.float16))
        bias0 = np.ascontiguousarray(np.broadcast_to(b0[rows], (B, G)).astype(np.float32))
        bias1 = np.ascontiguousarray(np.broadcast_to(b1[rows], (B, G)).astype(np.float32))
        woutt = np.ascontiguousarray(np.asarray(W_out, np.float32)[u].T.astype(np.float16))
        embx = np.ascontiguousarray(
            emb0[tidx[:, 1:STEPS]].transpose(1, 0, 2))  # [STEPS-1, B, G]
        in_maps.append({
            "embx": embx,
            "audio": np.ascontiguousarray(audio[8 * k: 8 * (k + 1)]),
            "tidx": tidx,
            "wproj": wproj_t,
            "bproj": bproj,
            "wh0t": wh0t, "wi0t": wi0t, "wi1t": wi1t, "wh1t": wh1t,
            "emb0": emb0, "bias0": bias0, "bias1": bias1,
            "woutt": woutt, "bout": np.ascontiguousarray(bout_f[u.start:u.stop]),
        })
    return in_maps


class _SpmdRunner:
    """Builds the sharded jit once; warm calls avoid re-tracing."""

    def __init__(self, nc, n_cores):
        import jax
        from jax.sharding import Mesh, PartitionSpec
        from jax.experimental.shard_map import shard_map
        from concourse.bass2jax import (_bass_exec_p, install_neuronx_cc_hook,
                                        partition_id_tensor)
        self._jax = jax
        install_neuronx_cc_hook()
        self.nc = nc
        self.n_cores = n_cores
        partition_name = (nc.partition_id_tensor.name
                          if nc.partition_id_tensor else None)
        in_names, out_names, out_avals, zero_outs = [], [], [], []
        for alloc in nc.m.functions[0].allocations:
            if not isinstance(alloc, mybir.MemoryLocationSet):
                continue
            name = alloc.memorylocations[0].name
            if alloc.kind == "ExternalInput":
                if name != partition_name:
                    in_names.append(name)
            elif alloc.kind == "ExternalOutput":
                out_names.append(name)
                shape = tuple(alloc.tensor_shape)
                dtype = mybir.dt.np(alloc.dtype)
                out_avals.append(jax.core.ShapedArray(shape, dtype))
                zero_outs.append(np.zeros(shape, dtype))
        self.in_names, self.out_names = in_names, out_names
        self.out_avals, self.zero_outs = out_avals, zero_outs
        n_params, n_outs = len(in_names), len(out_avals)
        all_in = list(in_names) + list(out_names)
        if partition_name is not None:
            all_in.append(partition_name)

        def _body(*args):
            operands = list(args)
            if partition_name is not None:
                operands.append(partition_id_tensor())
            outs = _bass_exec_p.bind(
                *operands, out_avals=tuple(out_avals), in_names=tuple(all_in),
                out_names=tuple(out_names), lowering_input_output_aliases=(),
                sim_require_finite=True, sim_require_nnan=True, nc=nc,
            )
            return tuple(outs)

        devices = jax.devices()[:n_cores]
        mesh = Mesh(np.asarray(devices), ("core",))
        in_specs = (PartitionSpec("core"),) * (n_params + n_outs)
        out_specs = (PartitionSpec("core"),) * n_outs
        self._fn = jax.jit(
            shard_map(_body, mesh=mesh, in_specs=in_specs,
                      out_specs=out_specs, check_rep=False),
            keep_unused=True,
        )
        self._zeros_dev = [
            jax.device_put(np.zeros((n_cores * z.shape[0], *z.shape[1:]), z.dtype))
            for z in zero_outs
        ]

    def prep_inputs(self, in_maps):
        return [
            self._jax.device_put(np.concatenate(
                [np.asarray(in_maps[c][n]) for c in range(self.n_cores)], axis=0))
            for n in self.in_names
        ]

    def run(self, concat_in):
        out = self._fn(*concat_in, *self._zeros_dev)
        self._jax.block_until_ready(out)
        return out

    def results(self, out_arrs):
        return [
            {n: np.asarray(out_arrs[i]).reshape(self.n_cores,
                                                *self.out_avals[i].shape)[c]
             for i, n in enumerate(self.out_names)}
            for c in range(self.n_cores)
        ]


_CACHE = {}


def get_runner(n_super=N_SUPER, dbg=False, **kw):
    key = (n_super, dbg, tuple(sorted(kw.items())))
    if key not in _CACHE:
        nc = bacc.Bacc("TRN2", target_bir_lowering=False, debug=False,
                       num_devices=N_CORES)
        _emit(nc, n_super, dbg=dbg, **kw)
        _CACHE[key] = _SpmdRunner(nc, N_CORES)
    return _CACHE[key]


def assemble(results, n_super=N_SUPER):
    steps = n_super - 2
    full = np.empty((B, steps, H), np.float32)
    for k in range(N_CORES):
        # per-core logits: [steps, U, B] -> [B, steps, U]
        full[:, :, 128 * k:128 * (k + 1)] = np.transpose(
            results[k]["logits"], (2, 0, 1))
    return full


def kernel(**inputs):
    in_maps = _prep_in_maps(**inputs)
    runner = get_runner()
    out = runner.run(runner.prep_inputs(in_maps))
    return assemble(runner.results(out))
